# revision 1
# baseline (speedup 1.0000x reference)
"""Quantized linear (dynamic per-tensor int8) on 8 TRN2 NeuronCores.

Reference semantics:
    x_q = round(x / s_x), s_x = max|x|/127   (per-tensor, round-half-even)
    w_q = round(w / s_w), s_w = max|w|/127
    out = (x_q @ w_q.T) * (s_x * s_w) + bias

Distribution: data-parallel over M (8 shards of 1024 rows), weight
replicated.  Each core scans a disjoint 1/8 of x (its own shard) and of w
for the local absmax; a 2-element AllReduce(max) collective produces the
global scales.  Quantized values are exact small integers, held in bf16
(ints <= 127 are exact in bf16), so the TensorE bf16 matmul with fp32 PSUM
accumulation reproduces the int8 GEMM exactly (sums stay far below 2^24).

Rounding uses the fp32 magic-number trick: RNE(round(v)) == (v + 1.5*2^23)
- 1.5*2^23 for |v| <= 2^22, matching jnp.round (half-to-even).

Host-side work is layout only: inputs are passed transposed (K-major) so
both matmul operands land in SBUF with K on the partition axis without any
on-device transposes; the output is computed as out^T (N on partitions) so
the bias add is a per-partition ScalarE bias, and the host transposes back.
"""

import numpy as np

from concourse import bacc, bass_isa
import concourse.bass_utils as bass_utils
import concourse.mybir as mybir
import concourse.tile as tile

P = 128
M, K, N = 8192, 4096, 4096
NCORES = 8
MLOC = M // NCORES  # 1024 rows of x per core
WS = N // NCORES  # 512 columns of wT scanned per core for absmax
MAGIC = float(np.float32(1.5 * 2**23))
MFREE = 512  # moving free dim per matmul (one fp32 PSUM bank)
NSTRIP = 128  # n-columns of w quantized per strip

F32 = mybir.dt.float32
BF16 = mybir.dt.bfloat16
AX = mybir.AxisListType
ALU = mybir.AluOpType
ACTF = mybir.ActivationFunctionType


def build_body(tc, xT, wT, wscanT, bias, outT, *, n_cores, mfree, nstrip):
    nc = tc.nc
    k, m_loc = xT.shape
    n = wT.shape[1]
    ws = wscanT.shape[1]
    kt_n = k // P
    assert k % P == 0 and n % nstrip == 0 and nstrip % P == 0 and m_loc % mfree == 0

    with (
        tc.tile_pool(name="const", bufs=1) as const,
        tc.tile_pool(name="stats", bufs=1) as stats,
        tc.tile_pool(name="xf", bufs=2) as xf_pool,
        tc.tile_pool(name="xfq", bufs=2) as xfq_pool,
        tc.tile_pool(name="xq", bufs=1) as xq_pool,
        tc.tile_pool(name="wf", bufs=2) as wf_pool,
        tc.tile_pool(name="wq", bufs=4) as wq_pool,
        tc.tile_pool(name="ob", bufs=4) as ob_pool,
        tc.tile_pool(name="ps", bufs=6, space="PSUM") as ps_pool,
        tc.tile_pool(name="dram", bufs=1, space="DRAM") as dram,
    ):
        # ---- bias, laid out bias[j*128+p] -> bias_sb[p, j] --------------
        bias_sb = const.tile([P, n // P], F32)
        nc.sync.dma_start(bias_sb[:], bias.rearrange("(nt p) -> p nt", p=P))

        xT3 = xT.rearrange("(c p) m -> p c m", p=P)
        wsT3 = wscanT.rearrange("(c p) m -> p c m", p=P)
        wT3 = wT.rearrange("(kt p) n -> p kt n", p=P)

        # Phase ordering is engine-FIFO-aware: every engine executes its own
        # program strictly in order, so instructions that wait on the w
        # collective must never precede (on the same engine) work that can
        # run earlier, and vice versa.  Global order below gives each engine
        # a stall-free projection.

        # ---- 1. w absmax scan (8 MiB) + collective #1 trigger -----------
        WCK, XCK = 4, 2  # k-tiles per scan chunk
        n_wsc, n_xsc = kt_n // WCK, kt_n // XCK
        wmax_cols = stats.tile([P, n_wsc], F32)
        for i in range(n_wsc):
            tw = xf_pool.tile([P, WCK, ws], F32, tag="wscan")
            nc.sync.dma_start(tw[:], wsT3[:, i * WCK : (i + 1) * WCK, :])
            nc.vector.tensor_reduce(
                wmax_cols[:, i : i + 1], tw[:], axis=AX.XY, op=ALU.max,
                apply_absolute_value=True,
            )
        wlmax = stats.tile([P, 1], F32)
        nc.vector.tensor_reduce(wlmax[:], wmax_cols[:], axis=AX.X, op=ALU.max)
        wgmax_p = stats.tile([P, 1], F32)
        nc.gpsimd.partition_all_reduce(
            wgmax_p[:], wlmax[:], channels=P, reduce_op=bass_isa.ReduceOp.max
        )
        wcc_in = dram.tile([1, 1], F32)
        wcc_out = dram.tile([1, 1], F32)
        nc.gpsimd.dma_start(wcc_in[:], wgmax_p[0:1, :])
        nc.gpsimd.collective_compute(
            "AllReduce", ALU.max, replica_groups=[list(range(n_cores))],
            ins=[wcc_in.opt()], outs=[wcc_out.opt()],
        )

        # ---- 2. x absmax scan (16 MiB), overlapping collective #1 -------
        xmax_cols = stats.tile([P, n_xsc], F32)
        for i in range(n_xsc):
            t = xf_pool.tile([P, XCK, m_loc], F32, tag="xf")
            nc.sync.dma_start(t[:], xT3[:, i * XCK : (i + 1) * XCK, :])
            nc.vector.tensor_reduce(
                xmax_cols[:, i : i + 1], t[:], axis=AX.XY, op=ALU.max,
                apply_absolute_value=True,
            )
        xlmax = stats.tile([P, 1], F32)
        nc.vector.tensor_reduce(xlmax[:], xmax_cols[:], axis=AX.X, op=ALU.max)

        # ---- 3. w scales from collective #1 -----------------------------
        wgmax = stats.tile([1, 1], F32)
        nc.gpsimd.dma_start(wgmax[:], wcc_out[:])
        wsc2 = stats.tile([1, 2], F32)
        wrec = stats.tile([1, 1], F32)
        nc.vector.reciprocal(wrec[:], wgmax[:])
        nc.vector.tensor_scalar(wsc2[:, 0:1], wrec[:], 127.0, None, op0=ALU.mult)
        nc.vector.tensor_scalar(
            wsc2[:, 1:2], wgmax[:], float(np.float32(1.0 / 127.0)), None,
            op0=ALU.mult,
        )
        wscb = const.tile([P, 2], F32)
        nc.gpsimd.partition_broadcast(wscb[:], wsc2[:])
        inv_sw = wscb[:, 0:1]
        s_w = wscb[:, 1:2]

        # ---- 4. x collective #2 trigger ---------------------------------
        xgmax_p = stats.tile([P, 1], F32)
        nc.gpsimd.partition_all_reduce(
            xgmax_p[:], xlmax[:], channels=P, reduce_op=bass_isa.ReduceOp.max
        )
        xcc_in = dram.tile([1, 1], F32)
        xcc_out = dram.tile([1, 1], F32)
        nc.gpsimd.dma_start(xcc_in[:], xgmax_p[0:1, :])
        nc.gpsimd.collective_compute(
            "AllReduce", ALU.max, replica_groups=[list(range(n_cores))],
            ins=[xcc_in.opt()], outs=[xcc_out.opt()],
        )

        # ---- 5. pre-quantize first w strips (needs only inv_sw) ---------
        n_strips = n // nstrip
        pre_q = min(4, n_strips)
        wqs = {}
        for s in range(pre_q):
            wf = wf_pool.tile([P, kt_n, nstrip], F32, tag="wf")
            nc.sync.dma_start(wf[:], wT3[:, :, s * nstrip : (s + 1) * nstrip])
            nc.scalar.activation(wf[:], wf[:], ACTF.Copy, bias=MAGIC, scale=inv_sw)
            wq = wq_pool.tile([P, kt_n, nstrip], BF16, tag="wq")
            nc.vector.tensor_scalar(wq[:], wf[:], MAGIC, None, op0=ALU.subtract)
            wqs[s] = wq

        # ---- 6. x scales from collective #2 -----------------------------
        xgmax = stats.tile([1, 1], F32)
        nc.gpsimd.dma_start(xgmax[:], xcc_out[:])
        xsc2 = stats.tile([1, 2], F32)
        xrec = stats.tile([1, 1], F32)
        nc.vector.reciprocal(xrec[:], xgmax[:])
        nc.vector.tensor_scalar(xsc2[:, 0:1], xrec[:], 127.0, None, op0=ALU.mult)
        nc.vector.tensor_scalar(
            xsc2[:, 1:2], xgmax[:], float(np.float32(1.0 / 127.0)), None,
            op0=ALU.mult,
        )
        xscb = const.tile([P, 2], F32)
        nc.gpsimd.partition_broadcast(xscb[:], xsc2[:])
        inv_sx = xscb[:, 0:1]
        out_sc = const.tile([P, 1], F32)
        nc.vector.tensor_tensor(out_sc[:], xscb[:, 1:2], s_w, op=ALU.mult)

        # ---- 7. quantize x shard -> resident bf16 (DVE) -----------------
        n_mh = m_loc // mfree
        QCK = 4  # k-tiles per quantize chunk
        xqs = [xq_pool.tile([P, kt_n, mfree], BF16, name=f"xq{h}") for h in range(n_mh)]
        for h in range(n_mh):
            for i in range(kt_n // QCK):
                xf = xfq_pool.tile([P, QCK, mfree], F32, tag="xfq")
                nc.sync.dma_start(
                    xf[:],
                    xT3[:, i * QCK : (i + 1) * QCK, h * mfree : (h + 1) * mfree],
                )
                nc.vector.tensor_scalar(
                    xf[:], xf[:], inv_sx, MAGIC, op0=ALU.mult, op1=ALU.add
                )
                nc.vector.tensor_scalar(
                    xqs[h][:, i * QCK : (i + 1) * QCK, :], xf[:], MAGIC, None,
                    op0=ALU.subtract,
                )

        # ---- 8. stream w strips: quantize (ACT+DVE), matmul, evict ------
        for s in range(n_strips):
            if s in wqs:
                wq = wqs[s]
            else:
                wf = wf_pool.tile([P, kt_n, nstrip], F32, tag="wf")
                nc.sync.dma_start(wf[:], wT3[:, :, s * nstrip : (s + 1) * nstrip])
                nc.scalar.activation(
                    wf[:], wf[:], ACTF.Copy, bias=MAGIC, scale=inv_sw
                )
                wq = wq_pool.tile([P, kt_n, nstrip], BF16, tag="wq")
                nc.vector.tensor_scalar(wq[:], wf[:], MAGIC, None, op0=ALU.subtract)
            for nt in range(nstrip // P):
                gn = s * nstrip + nt * P  # global n of this out^T row-tile
                for mh in range(m_loc // mfree):
                    ps = ps_pool.tile([P, mfree], F32)
                    for kt in range(kt_n):
                        nc.tensor.matmul(
                            ps[:],
                            wq[:, kt, nt * P : (nt + 1) * P],
                            xqs[mh][:, kt, :],
                            start=(kt == 0),
                            stop=(kt == kt_n - 1),
                        )
                    ob = ob_pool.tile([P, mfree], F32, tag="ob")
                    nc.vector.tensor_scalar(
                        ob[:], ps[:], out_sc[:], bias_sb[:, gn // P : gn // P + 1],
                        op0=ALU.mult, op1=ALU.add,
                    )
                    nc.gpsimd.dma_start(
                        outT[gn : gn + P, mh * mfree : (mh + 1) * mfree], ob[:]
                    )


def build_nc(m_loc=MLOC, k=K, n=N, ws=WS, n_cores=NCORES, mfree=MFREE, nstrip=NSTRIP):
    nc = bacc.Bacc("TRN2", target_bir_lowering=False, debug=False,
                   num_devices=n_cores)
    xT = nc.dram_tensor("xT", [k, m_loc], F32, kind="ExternalInput").ap()
    wT = nc.dram_tensor("wT", [k, n], F32, kind="ExternalInput").ap()
    wscanT = nc.dram_tensor("wscanT", [k, ws], F32, kind="ExternalInput").ap()
    bias = nc.dram_tensor("bias", [n], F32, kind="ExternalInput").ap()
    outT = nc.dram_tensor("outT", [n, m_loc], F32, kind="ExternalOutput").ap()
    with tile.TileContext(nc) as tc:
        build_body(tc, xT, wT, wscanT, bias, outT,
                   n_cores=n_cores, mfree=mfree, nstrip=nstrip)
    nc.compile()
    return nc


def make_in_maps(x, weight, bias, n_cores=NCORES):
    m_loc = x.shape[0] // n_cores
    ws = weight.shape[0] // n_cores
    wT = np.ascontiguousarray(weight.T)
    bias = np.ascontiguousarray(bias, dtype=np.float32)
    maps = []
    for c in range(n_cores):
        maps.append({
            "xT": np.ascontiguousarray(x[c * m_loc : (c + 1) * m_loc].T),
            "wT": wT,
            "wscanT": np.ascontiguousarray(weight[c * ws : (c + 1) * ws].T),
            "bias": bias,
        })
    return maps


_NC_CACHE = {}
LAST_RUN = None


def kernel(x, weight, bias, _trace=False):
    global LAST_RUN
    x = np.ascontiguousarray(np.asarray(x), dtype=np.float32)
    weight = np.ascontiguousarray(np.asarray(weight), dtype=np.float32)
    bias = np.asarray(bias, dtype=np.float32)
    if "full" not in _NC_CACHE:
        _NC_CACHE["full"] = build_nc()
    nc = _NC_CACHE["full"]
    in_maps = make_in_maps(x, weight, bias)
    res = bass_utils.run_bass_kernel_spmd(
        nc, in_maps, core_ids=list(range(NCORES)), trace=_trace
    )
    LAST_RUN = res
    out = np.empty((M, N), np.float32)
    for c in range(NCORES):
        out[c * MLOC : (c + 1) * MLOC, :] = res.results[c]["outT"].T
    return out



# revision 2
# speedup vs baseline: 1.0760x; 1.0760x over previous
"""Quantized linear (dynamic per-tensor int8) on 8 TRN2 NeuronCores.

Reference semantics:
    x_q = round(x / s_x), s_x = max|x|/127   (per-tensor, round-half-even)
    w_q = round(w / s_w), s_w = max|w|/127
    out = (x_q @ w_q.T) * (s_x * s_w) + bias

Distribution: data-parallel over M (8 shards of 1024 rows), weight
replicated.  Each core scans a disjoint 1/8 of x (its own shard) and of w
for the local absmax; a single 2-element AllReduce(max) collective produces
both global scales in one round trip.  Quantized values are exact small
integers held in fp16 (ints <= 2047 are exact in fp16), so the TensorE fp16
matmul with fp32 PSUM accumulation reproduces the int8 GEMM exactly (all
partial sums stay far below 2^24).

Rounding uses an fp16 magic: (v*inv_s + 1536) written to fp16 rounds the
fractional part half-to-even (ulp = 1 in [1024, 2048)), then an in-place
fp16 subtract of 1536 (2x DVE rate) recovers the integer, matching
jnp.round.

Scheduling notes (engine FIFOs execute in emission order):
  * All large f32 staging goes through ONE pool tag ("stg", 10 x 1 MiB
    ring).  The ring's WAR chain defers every re-read/prefetch DMA until
    the absmax-scan chunks it displaces have been consumed, so the scan
    (which gates the collective) gets full HBM bandwidth first, and the
    re-reads then land inside the collective's latency window.
  * One fused AllReduce instead of two staggered ones removes a full
    collective round trip plus the strict-FIFO GpSimd serialization
    between them.
  * The w-strip quantize for strip s+2 is emitted BEFORE the PSUM
    evacuations of strip s so the DVE FIFO never makes the TensorE wait.
"""

import numpy as np

from concourse import bacc, bass_isa
import concourse.bass_utils as bass_utils
import concourse.mybir as mybir
import concourse.tile as tile

P = 128
M, K, N = 8192, 4096, 4096
NCORES = 8
MLOC = M // NCORES  # 1024 rows of x per core
WS = N // NCORES  # 512 columns of wT scanned per core for absmax
MAGIC = 1536.0  # fp16 round-to-int magic: [1024,2048) has ulp 1
MFREE = 512  # moving free dim per matmul (one fp32 PSUM bank)
NSTRIP = 128  # n-columns of w quantized per strip
INV127 = float(np.float32(1.0 / 127.0))

F32 = mybir.dt.float32
F16 = mybir.dt.float16
AX = mybir.AxisListType
ALU = mybir.AluOpType
ACTF = mybir.ActivationFunctionType


def build_body(tc, xT, wT, wscanT, bias, outT, *, n_cores):
    nc = tc.nc
    k, m_loc = xT.shape
    n = wT.shape[1]
    kt_n = k // P  # 32
    n_strips = n // NSTRIP  # 32
    n_mh = m_loc // MFREE  # 2
    n_ck = kt_n // 4  # 8 quantize chunks of 4 k-tiles per mh

    with (
        tc.tile_pool(name="const", bufs=1) as const,
        tc.tile_pool(name="stats", bufs=1) as stats,
        tc.tile_pool(name="stage", bufs=10) as stage,
        tc.tile_pool(name="xq", bufs=1) as xq_pool,
        tc.tile_pool(name="wq", bufs=4) as wq_pool,
        tc.tile_pool(name="ob", bufs=4) as ob_pool,
        tc.tile_pool(name="ps", bufs=6, space="PSUM") as ps_pool,
        tc.tile_pool(name="dram", bufs=1, space="DRAM") as dram,
    ):
        # ---- bias, laid out bias[s*128+p] -> bias_sb[p, s] ---------------
        bias_sb = const.tile([P, n // P], F32)
        nc.sync.dma_start(bias_sb[:], bias.rearrange("(nt p) -> p nt", p=P))

        xT3 = xT.rearrange("(c p) m -> p c m", p=P)  # [128, 32, 1024]
        wsT3 = wscanT.rearrange("(c p) m -> p c m", p=P)  # [128, 32, 512]
        wT3 = wT.rearrange("(kt p) n -> p kt n", p=P)  # [128, 32, 4096]

        # ---- 1. absmax scans (ring allocs 0..23: run at full HBM BW) ----
        wmax_cols = stats.tile([P, 8], F32)
        for i in range(8):
            tw = stage.tile([P, 4, WS], F32, tag="stg", name=f"wsc{i}")
            nc.sync.dma_start(tw[:], wsT3[:, i * 4 : (i + 1) * 4, :])
            nc.vector.tensor_reduce(
                wmax_cols[:, i : i + 1], tw[:], axis=AX.XY, op=ALU.max,
                apply_absolute_value=True,
            )
        xmax_cols = stats.tile([P, 16], F32)
        for i in range(16):
            tx = stage.tile([P, 2, m_loc], F32, tag="stg", name=f"xsc{i}")
            nc.sync.dma_start(tx[:], xT3[:, i * 2 : (i + 1) * 2, :])
            nc.vector.tensor_reduce(
                xmax_cols[:, i : i + 1], tx[:], axis=AX.XY, op=ALU.max,
                apply_absolute_value=True,
            )
        lmax2 = stats.tile([P, 2], F32)
        nc.vector.tensor_reduce(lmax2[:, 0:1], wmax_cols[:], axis=AX.X, op=ALU.max)
        nc.vector.tensor_reduce(lmax2[:, 1:2], xmax_cols[:], axis=AX.X, op=ALU.max)

        # ---- 2. ONE fused collective: AllReduce(max) over [wmax, xmax] --
        gmax2 = stats.tile([P, 2], F32)
        nc.gpsimd.partition_all_reduce(
            gmax2[:, 0:1], lmax2[:, 0:1], channels=P,
            reduce_op=bass_isa.ReduceOp.max,
        )
        nc.gpsimd.partition_all_reduce(
            gmax2[:, 1:2], lmax2[:, 1:2], channels=P,
            reduce_op=bass_isa.ReduceOp.max,
        )
        cc_in = dram.tile([1, 2], F32)
        cc_out = dram.tile([1, 2], F32)
        nc.gpsimd.dma_start(cc_in[:], gmax2[0:1, :])
        nc.gpsimd.collective_compute(
            "AllReduce", ALU.max, replica_groups=[list(range(n_cores))],
            ins=[cc_in.opt()], outs=[cc_out.opt()],
        )

        # ---- 3. gated prefetch: ring WAR defers these behind the scans --
        # (DMA triggers only; consumed post-collective.)
        wf_tiles = {}  # (s, half) -> f32 stage tile [P, 16, 128]
        xre_tiles = {}  # (mh, ck) -> f32 stage tile [P, 4, 512]

        def load_wf(s):
            for h in range(2):
                t = stage.tile([P, 16, NSTRIP], F32, tag="stg", name=f"wf{s}_{h}")
                nc.sync.dma_start(
                    t[:],
                    wT3[:, h * 16 : (h + 1) * 16,
                        s * NSTRIP : (s + 1) * NSTRIP],
                )
                wf_tiles[(s, h)] = t

        def load_xre(mh, ck):
            t = stage.tile([P, 4, MFREE], F32, tag="stg", name=f"xr{mh}_{ck}")
            nc.sync.dma_start(
                t[:],
                xT3[:, ck * 4 : (ck + 1) * 4,
                    mh * MFREE : (mh + 1) * MFREE],
            )
            xre_tiles[(mh, ck)] = t

        load_wf(0)
        for ck in range(n_ck):
            load_xre(0, ck)
        load_wf(1)
        for ck in range(n_ck):
            load_xre(1, ck)
        load_wf(2)
        load_wf(3)

        # ---- 4. scales from the collective -------------------------------
        gsb = stats.tile([1, 2], F32)
        nc.gpsimd.dma_start(gsb[:], cc_out[:])
        wrec = stats.tile([1, 1], F32)
        xrec = stats.tile([1, 1], F32)
        s_w = stats.tile([1, 1], F32)
        s_x = stats.tile([1, 1], F32)
        sc4 = stats.tile([1, 4], F32)
        nc.vector.reciprocal(wrec[:], gsb[:, 0:1])
        nc.vector.reciprocal(xrec[:], gsb[:, 1:2])
        nc.vector.tensor_scalar(sc4[:, 0:1], wrec[:], 127.0, None, op0=ALU.mult)
        nc.vector.tensor_scalar(sc4[:, 1:2], xrec[:], 127.0, None, op0=ALU.mult)
        nc.vector.tensor_scalar(s_w[:], gsb[:, 0:1], INV127, None, op0=ALU.mult)
        nc.vector.tensor_scalar(s_x[:], gsb[:, 1:2], INV127, None, op0=ALU.mult)
        nc.vector.tensor_tensor(sc4[:, 2:3], s_w[:], s_x[:], op=ALU.mult)
        scb = const.tile([P, 4], F32)
        nc.gpsimd.partition_broadcast(scb[:], sc4[:])
        inv_sw = scb[:, 0:1]
        inv_sx = scb[:, 1:2]
        out_sc = scb[:, 2:3]

        # ---- 5. quantize helpers ----------------------------------------
        wq_tiles = {}

        def quant_w_strip(s):
            wq = wq_pool.tile([P, kt_n, NSTRIP], F16, tag="wq", name=f"wq{s}")
            for h in range(2):
                sl = wq[:, h * 16 : (h + 1) * 16, :]
                nc.scalar.activation(
                    sl, wf_tiles.pop((s, h))[:], ACTF.Copy,
                    bias=MAGIC, scale=inv_sw,
                )
                nc.vector.tensor_scalar(sl, sl, MAGIC, None, op0=ALU.subtract)
            wq_tiles[s] = wq

        xqs = [
            xq_pool.tile([P, kt_n, MFREE], F16, tag=f"xq{h}", name=f"xq{h}")
            for h in range(n_mh)
        ]

        def quant_x_chunk(mh, ck, on_act):
            sl = xqs[mh][:, ck * 4 : (ck + 1) * 4, :]
            src = xre_tiles.pop((mh, ck))[:]
            if on_act:
                nc.scalar.activation(sl, src, ACTF.Copy, bias=MAGIC, scale=inv_sx)
            else:
                nc.vector.tensor_scalar(
                    sl, src, inv_sx, MAGIC, op0=ALU.mult, op1=ALU.add
                )
            nc.vector.tensor_scalar(sl, sl, MAGIC, None, op0=ALU.subtract)

        # ---- 6. quantize prelude (strips 0-1, all of x) -----------------
        quant_x_chunk(0, 0, on_act=False)
        quant_w_strip(0)
        for ck in range(1, n_ck):
            quant_x_chunk(0, ck, on_act=(ck % 2 == 0))
        quant_w_strip(1)
        for ck in range(n_ck):
            quant_x_chunk(1, ck, on_act=(ck % 2 == 0))

        # ---- 7. stream: per strip s: prefetch s+4, quantize s+2, MM s ---
        for s in range(n_strips):
            if s + 4 < n_strips:
                load_wf(s + 4)
            if s + 2 < n_strips:
                quant_w_strip(s + 2)
            wq = wq_tiles.pop(s)
            for mh in range(n_mh):
                ps = ps_pool.tile([P, MFREE], F32)
                for kt in range(kt_n):
                    nc.tensor.matmul(
                        ps[:],
                        wq[:, kt, :],
                        xqs[mh][:, kt, :],
                        start=(kt == 0),
                        stop=(kt == kt_n - 1),
                    )
                ob = ob_pool.tile([P, MFREE], F32, tag="ob")
                nc.vector.tensor_scalar(
                    ob[:], ps[:], out_sc, bias_sb[:, s : s + 1],
                    op0=ALU.mult, op1=ALU.add,
                )
                nc.gpsimd.dma_start(
                    outT[s * NSTRIP : (s + 1) * NSTRIP,
                         mh * MFREE : (mh + 1) * MFREE],
                    ob[:],
                )


def build_nc(m_loc=MLOC, k=K, n=N, ws=WS, n_cores=NCORES):
    nc = bacc.Bacc("TRN2", target_bir_lowering=False, debug=False,
                   num_devices=n_cores)
    xT = nc.dram_tensor("xT", [k, m_loc], F32, kind="ExternalInput").ap()
    wT = nc.dram_tensor("wT", [k, n], F32, kind="ExternalInput").ap()
    wscanT = nc.dram_tensor("wscanT", [k, ws], F32, kind="ExternalInput").ap()
    bias = nc.dram_tensor("bias", [n], F32, kind="ExternalInput").ap()
    outT = nc.dram_tensor("outT", [n, m_loc], F32, kind="ExternalOutput").ap()
    with tile.TileContext(nc) as tc:
        build_body(tc, xT, wT, wscanT, bias, outT, n_cores=n_cores)
    nc.compile()
    return nc


def make_in_maps(x, weight, bias, n_cores=NCORES):
    m_loc = x.shape[0] // n_cores
    ws = weight.shape[0] // n_cores
    wT = np.ascontiguousarray(weight.T)
    bias = np.ascontiguousarray(bias, dtype=np.float32)
    maps = []
    for c in range(n_cores):
        maps.append({
            "xT": np.ascontiguousarray(x[c * m_loc : (c + 1) * m_loc].T),
            "wT": wT,
            "wscanT": np.ascontiguousarray(weight[c * ws : (c + 1) * ws].T),
            "bias": bias,
        })
    return maps


_NC_CACHE = {}
LAST_RUN = None


def kernel(x, weight, bias, _trace=False):
    global LAST_RUN
    x = np.ascontiguousarray(np.asarray(x), dtype=np.float32)
    weight = np.ascontiguousarray(np.asarray(weight), dtype=np.float32)
    bias = np.asarray(bias, dtype=np.float32)
    if "full" not in _NC_CACHE:
        _NC_CACHE["full"] = build_nc()
    nc = _NC_CACHE["full"]
    in_maps = make_in_maps(x, weight, bias)
    res = bass_utils.run_bass_kernel_spmd(
        nc, in_maps, core_ids=list(range(NCORES)), trace=_trace
    )
    LAST_RUN = res
    out = np.empty((M, N), np.float32)
    for c in range(NCORES):
        out[c * MLOC : (c + 1) * MLOC, :] = res.results[c]["outT"].T
    return out
